# revision 1
# baseline (speedup 1.0000x reference)
"""Trainium2 Bass kernel for nn_BandwidthPredictorNNHall.

Math: for each batch b (8 of them, one per NeuronCore) with particles
x [n=1024, d=4]:
    pilot_d = 1.0592 * std(x_d, ddof=1) * n^(-1/8)
    q = x / pilot,   K_ij = exp(-0.5 * |q_i - q_j|^2)
    s2_d = sum_ij K_ij ((q_jd - q_id)^2 - 1)
    s3_d = sum_ij K_ij (dx^3 - 3 dx)  == 0 exactly (odd under i<->j swap),
           so bandwidth2 is fp-cancellation noise in the reference
           (|bw2/bw1| ~ 6e-9) and is treated as 0.
With Mp = [1, p_1..p_4, p_1^2..p_4^2] (n x 9, RAW particle units), every sum
needed for s2 is an entry of V = Mp^T K Mp after a host-side 1/pilot^2
rescale:
    s2_d = ((V[0,5+d] + V[5+d,0] - 2 V[1+d,1+d]) / pilot_d^2 - V[0,0]) / sqrt(2pi)
The device computes V (9x9) and var (4) per batch; the host applies the
final ~30 scalar flops per batch.

Device pipeline per core (engine-balance driven; ScalarE's 1M exps are the
floor, everything else hides behind or around them):
  - One input DMA (each dma_start costs ~0.6us of queue time plus ~1.5us
    latency): a 3D-strided load mstatall [128, 8(tile), 4] in particle-major
    layout. The feature-major Gram operands are built from it with 8 small
    PE transposes instead of a second (slow, 4-byte-run) strided DMA.
  - sum(p) and sum(p^2) accumulate on the PE as two sequential matmul
    groups against a ones vector; tiny PE transposes move the results into
    row form. var/pinv2 = 1/(FACT^2 var) needs only a reciprocal -- no
    sqrt, so ScalarE runs just {Exp, Copy}: one activation-table set, one
    LoadActFuncSet, and it overlaps the DMA latency.
  - G_ij = q_i . q_j is computed as sum_d (p_id/pilot_d^2) p_jd (float32r
    streams at 1 cycle/row; even bf16-coarse rounding would move the final
    output < 1.5e-4, far below the reference's own fp32 noise).
  - K'' = exp(G - r_i/2): one [128,1024] ScalarE activation per row tile
    with per-partition bias, reading a 2-bank PSUM tile. r_i comes from a
    multiply + negated reduce against a PE-broadcast 0.5/pilot^2 row.
  - K'' is the true K column-scaled by c_j = e^{+r_j/2}; the scale is
    constant per column so it factors through P = K M and is cancelled
    exactly in stage V by MX = Mp . e^{-r/2}:
        PT  = Mp^T K''   (9-column weight loads, f32r stream, two PSUM
                          accumulation groups that chase the exp stream)
        P'' = PT^T per 128-col block (8 small PE transposes, one PSUM bank)
        V   = MX^T P'' = Mp^T K Mp
  - K symmetry makes the stored K'' row-tiles serve both orientations, so
    the [n,n] matrix is never transposed.
"""

import sys

sys.path.insert(0, "/opt/trn_rl_repo")

import numpy as np

_B, _N, _D = 8, 1024, 4
_P = 128
_NT = _N // _P  # 8 row tiles
_NM = 1 + 2 * _D  # 9 basis columns: [1, p, p^2]
_INV_SQRT_2PI = 1.0 / np.sqrt(2.0 * np.pi)
_RK = 0.282095
_FACT = 1.0592 * float(_N) ** (-1.0 / (4 + _D))

_NC = None  # compiled Bass module cache


def _build_kernel():
    import concourse.bass as bass  # noqa: F401
    import concourse.tile as tile
    from concourse import bacc, mybir
    from concourse.masks import make_identity

    f32 = mybir.dt.float32
    fr = mybir.dt.float32r
    Act = mybir.ActivationFunctionType
    Alu = mybir.AluOpType
    Ax = mybir.AxisListType

    nc = bacc.Bacc("TRN2", target_bir_lowering=False, debug=False, num_devices=_B)
    p_in = nc.dram_tensor("p", [_N, _D], f32, kind="ExternalInput")
    v_out = nc.dram_tensor("vout", [_NM, _NM], f32, kind="ExternalOutput")
    var_out = nc.dram_tensor("varout", [_D, 1], f32, kind="ExternalOutput")

    with tile.TileContext(nc) as tc:
        with (
            tc.tile_pool(name="singles", bufs=1) as singles,
            tc.tile_pool(name="psE", bufs=1, space="PSUM") as psE,
            tc.tile_pool(name="psV", bufs=1, space="PSUM") as psV,
            tc.tile_pool(name="psG", bufs=2, space="PSUM") as psG,
            tc.tile_pool(name="psPT", bufs=1, space="PSUM") as psPT,
        ):
            ident128 = singles.tile([_P, _P], f32, tag="identf")
            make_identity(nc, ident128)
            ident = ident128[0:_NM, 0:_NM]
            ones128 = singles.tile([_P, 1], f32, tag="ones128")
            nc.vector.memset(ones128, 1.0)
            ones_row = singles.tile([1, _P], f32, tag="ones_row")
            nc.vector.memset(ones_row, 1.0)
            onesN = singles.tile([_P, 1], f32, tag="onesN")
            nc.vector.memset(onesN, 1.0 / float(_N) ** 0.5)
            # dummy Exp so the activation-table load runs during the DMA wait
            warm = singles.tile([1, 1], f32, tag="warm")
            nc.scalar.activation(out=warm, in_=ones128[0:1, 0:1], func=Act.Exp)

            # ---- two input DMAs: particle-major tiles + feature-major rows
            mstatall = singles.tile([_P, _NT, _D], f32, tag="mstatall")
            nc.sync.dma_start(
                out=mstatall, in_=p_in[:].rearrange("(c i) d -> i c d", c=_NT)
            )
            msqall = singles.tile([_P, _NT, _D], f32, tag="msqall")
            nc.vector.tensor_mul(msqall, mstatall, mstatall)

            # ---- stats on the PE: two sequential accumulation groups
            # (sum p, then sum p^2), each copied out and transposed to a
            # row so the var chain runs at partition 0
            sv4 = []
            for g, (src, rv) in enumerate(((mstatall, onesN), (msqall, ones128))):
                st4 = psE.tile([_D, 1], f32, tag="early")
                for c in range(_NT):
                    nc.tensor.matmul(
                        st4, lhsT=src[:, c, :], rhs=rv,
                        start=(c == 0), stop=(c == _NT - 1),
                    )
                sv = singles.tile([_D, 1], f32, tag=f"sv4_{g}")
                nc.vector.tensor_copy(sv, st4)
                sv4.append(sv)
            # den = sum(p^2) - sum(p)^2/n = (n-1) var; phcol = 0.5/pilot^2
            den = singles.tile([_D, 1], f32, tag="den")
            nc.vector.tensor_mul(den, sv4[0], sv4[0])
            nc.vector.tensor_sub(den, sv4[1], den)
            var_t = singles.tile([_D, 1], f32, tag="var_t")
            nc.vector.tensor_scalar_mul(var_t, den, 1.0 / (_N - 1))
            nc.sync.dma_start(out=var_out[:], in_=var_t)
            denf = singles.tile([_D, 1], f32, tag="denf")
            nc.vector.tensor_scalar_mul(
                denf, den, 2.0 * _FACT * _FACT / (_N - 1)
            )
            phcol = singles.tile([_D, 1], f32, tag="phcol")
            nc.vector.reciprocal(phcol, denf)

            # QTr = p in feature-major f32r via 8 PE transposes of the
            # tile-major data (no second DMA); Qs = QTr * 2*phcol
            QTr = singles.tile([_D, _N], fr, tag="qtr")
            for c in range(_NT):
                cs = slice(c * _P, (c + 1) * _P)
                ps_q = psG.tile([_D, _P], f32, tag="psg")
                nc.tensor.transpose(ps_q, mstatall[:, c, :], ident128)
                nc.vector.tensor_copy(QTr[:, cs], ps_q)
            Qs = singles.tile([_D, _N], fr, tag="qs")
            nc.vector.tensor_scalar(
                out=Qs, in0=QTr, scalar1=phcol, scalar2=2.0,
                op0=Alu.mult, op1=Alu.mult,
            )

            # 0.5/pilot^2 as a row + broadcast to [128,4] via rank-1 PE
            # outer product (for the r_i reductions)
            ps_pr = psE.tile([1, _D], f32, tag="early")
            nc.tensor.transpose(ps_pr, phcol, ident[0:_D, 0:_D])
            ph_r = singles.tile([1, _D], f32, tag="ph_r")
            nc.vector.tensor_copy(ph_r, ps_pr)
            ps_bc = psE.tile([_P, _D], f32, tag="early")
            nc.tensor.matmul(ps_bc, lhsT=ones_row, rhs=ph_r, start=True, stop=True)
            bc_sb = singles.tile([_P, _D], f32, tag="bc_sb")
            nc.vector.tensor_copy(bc_sb, ps_bc)

            # ---- exp bias nhall[:, c] = -r/2 = -sum_d p^2 * (0.5/pilot^2)
            nhall = singles.tile([_P, _NT], f32, tag="nhall")
            scr = singles.tile([_P, _NT, _D], f32, tag="scr")
            for c in range(_NT):
                nc.vector.tensor_mul(scr[:, c, :], msqall[:, c, :], bc_sb)
                nc.vector.tensor_reduce(
                    out=nhall[:, c : c + 1], in_=scr[:, c, :],
                    axis=Ax.X, op=Alu.add, negate=True,
                )

            # ---- Mp tiles (f32r, PT-stage weights) built in strided copies;
            # MX = Mp . e^{-r/2} per tile
            mtall = singles.tile([_P, _NT, _NM], fr, tag="mtall")
            for c in range(_NT):
                nc.vector.tensor_copy(mtall[:, c, 0:1], ones128)
            nc.vector.tensor_copy(mtall[:, :, 1 : 1 + _D], mstatall)
            nc.vector.tensor_copy(mtall[:, :, 1 + _D : _NM], msqall)
            cneg = singles.tile([_P, _NT], f32, tag="cneg")
            nc.scalar.activation(out=cneg, in_=nhall, func=Act.Exp)
            mxall = singles.tile([_P, _NT, _NM], f32, tag="mxall")
            for c in range(_NT):
                nc.vector.tensor_scalar_mul(
                    mxall[:, c, :], mtall[:, c, :], cneg[:, c : c + 1]
                )

            # ---- main stream: per row tile, two f32r Gram matmuls into a
            # 2-bank PSUM tile, one [128,1024] Exp, then the tile's PT
            # contributions (both j-half accumulation groups chase the exps)
            KT = singles.tile([_P, _NT, _N], fr, tag="kt")
            pspt = psPT.tile([_NM, 2, 512], f32, tag="pspt")
            for ir in range(_NT):
                irs = slice(ir * _P, (ir + 1) * _P)
                psg = psG.tile([_P, 2, 512], f32, tag="psg")
                for jh in range(2):
                    js = slice(jh * 512, (jh + 1) * 512)
                    nc.tensor.matmul(
                        psg[:, jh, :],
                        lhsT=Qs[:, irs],
                        rhs=QTr[:, js],
                        start=True, stop=True,
                    )
                nc.scalar.activation(
                    out=KT[:, ir, :],
                    in_=psg.rearrange("p a b -> p (a b)"),
                    func=Act.Exp,
                    bias=nhall[:, ir : ir + 1],
                )
                for jh in range(2):
                    js = slice(jh * 512, (jh + 1) * 512)
                    nc.tensor.matmul(
                        pspt[:, jh, :],
                        lhsT=mtall[:, ir, :],
                        rhs=KT[:, ir, js],
                        start=(ir == 0), stop=(ir == _NT - 1),
                        skip_group_check=True,
                    )

            # ---- PT out of PSUM, P'' = PT^T per block into one PSUM bank,
            # V = MX^T P''
            pts = singles.tile([_NM, _N], f32, tag="pts")
            nc.vector.tensor_copy(pts[:, 0:512], pspt[:, 0, :])
            nc.vector.tensor_copy(pts[:, 512:1024], pspt[:, 1, :])
            psp2 = psE.tile([_P, _NT, _NM], f32, tag="early")
            for r in range(_NT):
                nc.tensor.transpose(
                    psp2[:, r, :], pts[:, r * _P : (r + 1) * _P], ident
                )
            prall = singles.tile([_P, _NT, _NM], f32, tag="prall")
            nc.vector.tensor_copy(prall, psp2)
            psv = psV.tile([_NM, _NM], f32, tag="psv")
            for r in range(_NT):
                nc.tensor.matmul(
                    psv, lhsT=mxall[:, r, :], rhs=prall[:, r, :],
                    start=(r == 0), stop=(r == _NT - 1),
                )
            Vt = singles.tile([_NM, _NM], f32, tag="vt")
            nc.vector.tensor_copy(Vt, psv)
            nc.sync.dma_start(out=v_out[:], in_=Vt)

    nc.compile()
    return nc


def _get_nc():
    global _NC
    if _NC is None:
        _NC = _build_kernel()
    return _NC


def finalize(V, var):
    """Host-side tail: V [9,9] (raw-p units), var [4] -> bandwidth [4]."""
    V = V.astype(np.float64)
    var = var.astype(np.float64).reshape(_D)
    pilot = _FACT * np.sqrt(var)
    d = np.arange(_D)
    s2 = (
        (V[0, 5 + d] + V[5 + d, 0] - 2.0 * V[1 + d, 1 + d]) / pilot**2 - V[0, 0]
    ) * _INV_SQRT_2PI
    denom = _N * (_N - 1)
    I2 = s2 / pilot**5 / denom
    J1 = _RK / I2
    base = J1 / _N
    return (np.sign(base) * np.abs(base) ** 0.2).astype(np.float32)


def kernel(particles, weights=None, **_unused):
    from concourse.bass_utils import run_bass_kernel_spmd

    particles = np.ascontiguousarray(np.asarray(particles), dtype=np.float32)
    assert particles.shape == (_B, _N, _D), particles.shape

    nc = _get_nc()
    in_maps = [{"p": particles[c]} for c in range(_B)]
    res = run_bass_kernel_spmd(nc, in_maps, list(range(_B)))

    out = np.empty((_B, _D), np.float32)
    for c in range(_B):
        out[c] = finalize(res.results[c]["vout"], res.results[c]["varout"])
    return out



# revision 23
# speedup vs baseline: 1.1280x; 1.1280x over previous
"""Trainium2 Bass kernel for nn_BandwidthPredictorNNHall.

Math: for each batch b (8 of them, one per NeuronCore) with particles
x [n=1024, d=4]:
    pilot_d = 1.0592 * std(x_d, ddof=1) * n^(-1/8)
    q = x / pilot,   K_ij = exp(-0.5 * |q_i - q_j|^2)
    s2_d = sum_ij K_ij ((q_jd - q_id)^2 - 1)
    s3 terms are exactly 0 by antisymmetry -> bandwidth2 treated as 0.

K is symmetric, so only the upper-triangular block half is computed:
row tile ir (128 rows) covers columns [128*ir, 1024).  Diagonal blocks
are half-weighted by accumulating -ln2 into their Gram PSUM region (a
tiny bf16 rank-1 matmul) so that W + W^T == Mp^T K Mp exactly, where
    W = sum_{block i<=j} Mp_i^T K''_{ij} Mp_j (with the usual e^{r/2}
column rescale cancelled by MX = Mp.e^{-r/2} in the V stage).

Device pipeline per core:
  - DMA 1: particle-major [128, 8, 4]; DMA 2: feature-major [4, 1024]
    (the Gram moving operand; costs no compute engine).
  - Stats on PE (one [8,1] PSUM tile, pre-scaled ones so the var chain
    is mult/sub/reciprocal only -> phcol2 = 1/pilot^2 = 1/denf).
  - 8 PE transposes build raw q^T in PSUM; one ScalarE Copy activation
    with per-partition scale phcol2 produces the scaled weights qsc.
  - Main loop (upper triangle, ascending): Gram chunks (f32r) -> one
    Exp activation per tile with bias -r_i/2 -> bf16 KT -> PT chunks
    (bf16, 1 cycle/row at any width) accumulating Mp^T K'' into a
    nested [9, 1024] PSUM region.  PT column block ir is final right
    after tile ir, so its V-stage (copy out, transpose, V-matmul into
    psv) pipelines into the loop 1-2 tiles behind.
  - Output: one [10, 9] DMA: rows 0-8 = W, row 9 cols 0-3 = pilot^2.
Host applies ~30 scalar flops per batch (finalize).
"""

import sys

sys.path.insert(0, "/opt/trn_rl_repo")

import numpy as np

_B, _N, _D = 8, 1024, 4
_P = 128
_NT = _N // _P  # 8 row tiles
_NM = 1 + 2 * _D  # 9 basis columns: [1, p, p^2]
_INV_SQRT_2PI = 1.0 / np.sqrt(2.0 * np.pi)
_RK = 0.282095
_FACT = 1.0592 * float(_N) ** (-1.0 / (4 + _D))

# column extents per row tile (upper triangle) and KT storage offsets
_C = [(_N - _P * ir) for ir in range(_NT)]
_OFF = [sum(_C[:ir]) for ir in range(_NT)]
_KTOT = sum(_C)  # 4608

_NC = None  # compiled Bass module cache


def _build_kernel():
    import concourse.bass as bass  # noqa: F401
    import concourse.tile as tile
    from concourse import bacc, mybir
    from concourse.masks import make_identity

    f32 = mybir.dt.float32
    fr = mybir.dt.float32r
    bf16 = mybir.dt.bfloat16
    Act = mybir.ActivationFunctionType
    Alu = mybir.AluOpType
    Ax = mybir.AxisListType

    # split -ln2 across the two bf16 rank-1 factors so the product is
    # ln2 to ~2^-16 relative
    import ml_dtypes

    _lx = float(np.abs(np.sqrt(np.log(2.0))).astype(ml_dtypes.bfloat16))
    _ly = float(np.array(np.log(2.0) / _lx, np.float32).astype(ml_dtypes.bfloat16))

    nc = bacc.Bacc("TRN2", target_bir_lowering=False, debug=False, num_devices=_B)
    p_in = nc.dram_tensor("p", [_N, _D], f32, kind="ExternalInput")
    w_out = nc.dram_tensor("wout", [_NM, 16], f32, kind="ExternalOutput")

    with tile.TileContext(nc) as tc:
        with (
            tc.tile_pool(name="singles", bufs=1) as singles,
            tc.tile_pool(name="psS", bufs=1, space="PSUM") as psS,
            tc.tile_pool(name="psG", bufs=1, space="PSUM") as psG,
            tc.tile_pool(name="psPT", bufs=1, space="PSUM") as psPT,
        ):
            ident128 = singles.tile([_P, _P], f32, tag="identf")
            make_identity(nc, ident128)
            ones128 = singles.tile([_P, 1], f32, tag="ones128")
            nc.vector.memset(ones128, 1.0)
            # pre-scaled ones so den comes out as denf = pilot^2 directly:
            #   m = (sum p * a)^2, denf = sum p^2 * b - m
            #   b = FACT^2/(n-1), a = FACT/sqrt(n(n-1))
            bconst = _FACT * _FACT / (_N - 1)
            aconst = _FACT / np.sqrt(float(_N) * (_N - 1))
            onesA = singles.tile([_P, 1], f32, tag="onesA")
            nc.vector.memset(onesA, aconst)
            onesB = singles.tile([_P, 1], f32, tag="onesB")
            nc.vector.memset(onesB, bconst)
            halfrow = singles.tile([1, _P], f32, tag="halfrow")
            nc.vector.memset(halfrow, 0.5)
            # bf16 rank-1 factors for the -ln2 diagonal half-weighting
            lnrow = singles.tile([1, _P], bf16, tag="lnrow")
            nc.vector.memset(lnrow, -_lx)
            lyrow = singles.tile([1, _P], bf16, tag="lyrow")
            nc.vector.memset(lyrow, _ly)
            # dummy Exp so the activation-table load runs during the DMA wait
            warm = singles.tile([1, 1], f32, tag="warm")
            nc.scalar.activation(out=warm, in_=ones128[0:1, 0:1], func=Act.Exp)

            # ---- input DMAs: particle-major tiles + feature-major stream
            mstatall = singles.tile([_P, _NT, _D], f32, tag="mstatall")
            nc.sync.dma_start(
                out=mstatall,
                in_=p_in[:].rearrange("(c i) d -> i c d", c=_NT),
            )
            qraw = singles.tile([_D, _N], fr, tag="qraw")

            msqall = singles.tile([_P, _NT, _D], f32, tag="msqall")
            nc.vector.tensor_mul(msqall, mstatall, mstatall)

            # ---- small PSUM tiles packed into two bank tiles
            smallA = psS.tile([_P, 512], f32, tag="smallA")
            smallB = psS.tile([_P, 512], f32, tag="smallB")
            # ---- stats on the PE: rows 0-3 = sum(p)*a / sum(p^2)*b
            st8 = smallA[0:_D, 0:3]
            for c in range(_NT):
                nc.tensor.matmul(
                    st8[:, 0:1], lhsT=mstatall[:, c, :], rhs=onesA,
                    start=(c == 0), stop=(c == _NT - 1),
                    skip_group_check=True,
                )
            for c in range(_NT):
                nc.tensor.matmul(
                    st8[:, 2:3], lhsT=msqall[:, c, :], rhs=onesB,
                    start=(c == 0), stop=(c == _NT - 1),
                    skip_group_check=True,
                )

            # ---- 8 PE transposes: raw q^T into PSUM [4, 1024]
            # (goes into the gram ring, slot 0; reused by gram tile 1)
            psq = psG.tile([_D, _N], f32, tag="g", bufs=2)
            for c in range(_NT):
                nc.tensor.transpose(
                    psq[:, c * _P : (c + 1) * _P], mstatall[:, c, :], ident128
                )

            # ---- var chain on DVE reading PSUM directly:
            # denf = pilot^2, phcol2 = 1/pilot^2
            stsb = singles.tile([_D, 3], f32, tag="stsb")
            nc.vector.tensor_copy(stsb, st8)
            mm = singles.tile([_D, 1], f32, tag="mm")
            nc.vector.tensor_mul(mm, stsb[:, 0:1], stsb[:, 0:1])
            denf = singles.tile([_D, 1], f32, tag="denf")
            nc.vector.tensor_sub(denf, stsb[:, 2:3], mm)
            phcol2 = singles.tile([_D, 1], f32, tag="phcol2")
            nc.vector.reciprocal(phcol2, denf)

            # ---- scaled weights qsc = q^T * phcol2 (per-partition scale)
            # on ScalarE (idle until the first exp); split so tile 0's
            # weights are ready early
            qsc = singles.tile([_D, _N], fr, tag="qsc")
            nc.scalar.activation(
                out=qsc[:, 0:_P], in_=psq[:, 0:_P], func=Act.Copy, scale=phcol2
            )
            nc.scalar.activation(
                out=qsc[:, _P:_N], in_=psq[:, _P:_N], func=Act.Copy, scale=phcol2
            )

            # ---- 0.5/pilot^2 broadcast [128, 4] via PE rank-1:
            # transpose phcol2 to a row, outer-product with 0.5-row
            ph_ps = smallA[0:1, 16:20]
            nc.tensor.matmul(
                ph_ps, lhsT=phcol2, rhs=ident128[0:_D, 0:_D],
                is_transpose=True, skip_group_check=True,
            )
            # pilot^2 row for the output (row 9 of w_out)
            denf_ps = smallA[0:1, 20:24]
            nc.tensor.matmul(
                denf_ps, lhsT=denf, rhs=ident128[0:_D, 0:_D],
                is_transpose=True, skip_group_check=True,
            )
            ph_r = singles.tile([1, _D], f32, tag="ph_r")
            nc.vector.tensor_copy(ph_r, ph_ps)
            bc_ps = smallA[:, 32:36]
            nc.tensor.matmul(
                bc_ps, lhsT=halfrow, rhs=ph_r,
                start=True, stop=True, skip_group_check=True,
            )
            bc_sb = singles.tile([_P, _D], f32, tag="bc_sb")
            nc.vector.tensor_copy(bc_sb, bc_ps)

            # ---- raw feature-major stream: two DVE copies out of psq
            # (f32r-rounded, as the fp32r Gram requires)
            nc.vector.tensor_copy(qraw[:, 0:512], psq[:, 0:512])

            # ---- exp bias nhall[:, c] = -r/2 = -sum_d p^2 * (0.5/pilot^2)
            scr = singles.tile([_P, _NT, _D], f32, tag="scr")
            nc.vector.tensor_mul(
                scr, msqall,
                bc_sb.rearrange("p (o d) -> p o d", o=1).broadcast_to([_P, _NT, _D]),
            )
            nhall = singles.tile([_P, _NT], f32, tag="nhall")
            nc.vector.tensor_reduce(
                out=nhall.rearrange("p (c o) -> p c o", o=1), in_=scr,
                axis=Ax.X, op=Alu.add, negate=True,
            )
            nc.vector.tensor_copy(qraw[:, 512:_N], psq[:, 512:_N])

            # ---- output staging tile: W in cols 0:9, pilot^2 in row 0
            # cols 12:16
            w_sb = singles.tile([_NM, 16], f32, tag="w_sb")
            nc.vector.tensor_copy(w_sb[0:1, 12:16], denf_ps)

            # ---- cneg = e^{-r/2} on ScalarE (fits before the first exp)
            cneg = singles.tile([_P, _NT], f32, tag="cneg")
            nc.scalar.activation(out=cneg, in_=nhall, func=Act.Exp)

            # ---- Mp tiles: bf16 for the PT stage, f32r * e^{-r/2} for V
            mtall = singles.tile([_P, _NT, _NM], bf16, tag="mtall")
            nc.vector.memset(mtall[:, :, 0:1], 1.0)
            nc.vector.tensor_copy(mtall[:, :, 1 : 1 + _D], mstatall)
            nc.vector.tensor_copy(mtall[:, :, 1 + _D : _NM], msqall)
            mp9 = singles.tile([_P, _NT, _NM], fr, tag="mp9")
            nc.vector.tensor_copy(
                mp9[:, :, 0:1],
                ones128.rearrange("p (o d) -> p o d", o=1).broadcast_to([_P, _NT, 1]),
            )
            nc.vector.tensor_copy(mp9[:, :, 1 : 1 + _D], mstatall)
            nc.vector.tensor_copy(mp9[:, :, 1 + _D : _NM], msqall)
            mxall = singles.tile([_P, _NT, _NM], fr, tag="mxall")

            # ---- main loop: upper-triangle row tiles, ascending.
            # KT stored bf16, tile ir at column offset _OFF[ir].
            ktall = singles.tile([_P, _KTOT], bf16, tag="ktall")
            pspt = psPT.tile([_NM, _N], f32, tag="pspt")
            prall = singles.tile([_P, _NT, _NM + 1], fr, tag="prall")
            nc.vector.tensor_copy(
                prall[:, :, _NM : _NM + 1],
                ones128.rearrange("p (o d) -> p o d", o=1).broadcast_to([_P, _NT, 1]),
            )
            psv = smallA[0:_NM, 48 : 48 + _NM + 1]

            def gram(ir):
                """Gram chunks for tile ir into a fresh gram-ring slot."""
                lo = _P * ir
                g = psG.tile([_P, _N], f32, tag="g", bufs=2, name=f"g{ir}")
                if lo < 512:
                    nc.tensor.matmul(
                        g[:, lo:512], lhsT=qsc[:, lo : lo + _P],
                        rhs=qraw[:, lo:512], start=True, stop=False,
                        skip_group_check=True,
                    )
                    nc.tensor.matmul(
                        g[:, 512:_N], lhsT=qsc[:, lo : lo + _P],
                        rhs=qraw[:, 512:_N], start=True, stop=True,
                        skip_group_check=True,
                    )
                else:
                    nc.tensor.matmul(
                        g[:, lo:_N], lhsT=qsc[:, lo : lo + _P],
                        rhs=qraw[:, lo:_N], start=True, stop=False,
                        skip_group_check=True,
                    )
                # -ln2 into the diagonal block (half-weighting)
                nc.tensor.matmul(
                    g[:, lo : lo + _P], lhsT=lnrow, rhs=lyrow,
                    start=False, stop=True, skip_group_check=True,
                )
                return g

            def exp_tile(ir, g):
                lo = _P * ir
                if ir == 0:
                    nc.scalar.activation(
                        out=ktall[:, 0:512], in_=g[:, 0:512],
                        func=Act.Exp, bias=nhall[:, 0:1],
                    )
                    nc.scalar.activation(
                        out=ktall[:, 512:_N], in_=g[:, 512:_N],
                        func=Act.Exp, bias=nhall[:, 0:1],
                    )
                else:
                    nc.scalar.activation(
                        out=ktall[:, _OFF[ir] : _OFF[ir] + _C[ir]],
                        in_=g[:, lo:_N],
                        func=Act.Exp, bias=nhall[:, ir : ir + 1],
                    )

            def pt_tile(ir):
                """PT chunks: pspt[:, 128*ir:1024] += Mp_ir^T KT_ir."""
                lo = _P * ir
                off = _OFF[ir]
                if lo < 512:
                    nc.tensor.matmul(
                        pspt[:, lo:512], lhsT=mtall[:, ir, :],
                        rhs=ktall[:, off : off + (512 - lo)],
                        start=(ir == 0), stop=(ir == 3),
                        skip_group_check=True,
                    )
                    nc.tensor.matmul(
                        pspt[:, 512:_N], lhsT=mtall[:, ir, :],
                        rhs=ktall[:, off + (512 - lo) : off + _C[ir]],
                        start=(ir == 0), stop=(ir == _NT - 1),
                        skip_group_check=True,
                    )
                else:
                    nc.tensor.matmul(
                        pspt[:, lo:_N], lhsT=mtall[:, ir, :],
                        rhs=ktall[:, off : off + _C[ir]],
                        start=False, stop=(ir == _NT - 1),
                        skip_group_check=True,
                    )

            ptsb = [None] * _NT  # [9, 128] copies of finished PT blocks

            def pts_copy(ir):
                t = singles.tile([_NM, _P], f32, tag=f"pts{ir}", name=f"pts{ir}")
                nc.vector.tensor_copy(t, pspt[:, _P * ir : _P * (ir + 1)])
                ptsb[ir] = t

            psp2 = [None] * _NT

            def transp_blk(ir):
                t = smallB[:, 16 * (ir % 2) : 16 * (ir % 2) + _NM]
                nc.tensor.matmul(
                    t, lhsT=ptsb[ir], rhs=ident128[0:_NM, 0:_NM],
                    is_transpose=True, skip_group_check=True,
                )
                psp2[ir] = t

            def prb_copy(ir):
                nc.vector.tensor_copy(prall[:, ir, 0:_NM], psp2[ir])

            def v_mm(ir):
                nc.tensor.matmul(
                    psv, lhsT=mxall[:, ir, :], rhs=prall[:, ir, :],
                    start=(ir == 0), stop=(ir == _NT - 1),
                    skip_group_check=True,
                )

            g = gram(0)
            for ir in range(_NT):
                exp_tile(ir, g)
                if ir + 1 < _NT:
                    g = gram(ir + 1)
                pt_tile(ir)
                if ir >= 1:
                    transp_blk(ir - 1)
                if ir >= 2:
                    v_mm(ir - 2)
                pts_copy(ir)
                if ir >= 1:
                    prb_copy(ir - 1)
                nc.vector.tensor_scalar_mul(
                    mxall[:, ir, :], mp9[:, ir, :], cneg[:, ir : ir + 1]
                )

            transp_blk(_NT - 1)
            v_mm(_NT - 2)
            pts_copy_done = None  # noqa: F841
            prb_copy(_NT - 1)
            v_mm(_NT - 1)

            nc.vector.tensor_copy(w_sb[:, 0:_NM], psv[:, 0:_NM])
            nc.sync.dma_start(out=w_out[:], in_=w_sb)

    nc.compile()
    return nc


def _get_nc():
    global _NC
    if _NC is None:
        _NC = _build_kernel()
    return _NC


def finalize(w10):
    """Host-side tail: w10 [9, 16]: cols 0-8 = W (upper-tri half of
    Mp^T K Mp, diag blocks half-weighted), row 0 cols 12-16 = pilot^2."""
    W = w10[:, 0:_NM].astype(np.float64)
    pilot2 = w10[0, 12 : 12 + _D].astype(np.float64)
    d = np.arange(_D)
    g = 2.0 * (W[0, 5 + d] + W[5 + d, 0] - 2.0 * W[1 + d, 1 + d])
    v00 = 2.0 * W[0, 0]
    s2 = (g / pilot2 - v00) * _INV_SQRT_2PI
    denom = _N * (_N - 1)
    pilot5 = pilot2**2 * np.sqrt(pilot2)
    I2 = s2 / pilot5 / denom
    J1 = _RK / I2
    base = J1 / _N
    return (np.sign(base) * np.abs(base) ** 0.2).astype(np.float32)


def kernel(particles, weights=None, **_unused):
    from concourse.bass_utils import run_bass_kernel_spmd

    particles = np.ascontiguousarray(np.asarray(particles), dtype=np.float32)
    assert particles.shape == (_B, _N, _D), particles.shape

    nc = _get_nc()
    in_maps = [{"p": particles[c]} for c in range(_B)]
    res = run_bass_kernel_spmd(nc, in_maps, list(range(_B)))

    out = np.empty((_B, _D), np.float32)
    for c in range(_B):
        out[c] = finalize(res.results[c]["wout"])
    return out
